# revision 1
# baseline (speedup 1.0000x reference)
"""Trainium2 Bass kernel for nn_EncoderBlock (dense transformer block).

Reference computation (fp32, S=2048 B=2 D=1024 H=16 dh=64 F=4096):
    q,k,v = x@Wq+bq, x@Wk+bk, x@Wv+bv          (per-head split, dh=64)
    attn  = softmax(q k^T / sqrt(dh)) v         (full S x S scores)
    o     = attn-merge @ Wo + bo
    x1    = LN(x + o; g1,b1)
    out   = LN(x1 + relu(x1@W1+bb1)@W2+bb2; g2,b2)

Sharding: sequence-parallel over 8 cores. Each core owns 256 seq positions
(x 2 batches = 512 tokens) end-to-end; K/V are computed redundantly on every
core (cheaper than an on-chip AllGather at this size). All matmuls run in
bf16 with fp32 PSUM accumulation.

Attention trick: scores are computed TRANSPOSED (S^T[tk,q], lhsT=K^T chunk,
rhs=Q^T chunk) so that exp(S^T) feeds the PV matmul directly as the moving
operand with token-major V as the stationary one -- no P transposes. Softmax
max-subtraction is skipped (scores have |s| < ~10, exp cannot overflow);
row sums come from a ones-vector matmul and are divided out of o^T via a
gpsimd partition-broadcast of the reciprocal row.
"""

import numpy as np
import ml_dtypes

import concourse.bass as bass
import concourse.mybir as mybir
import concourse.tile as tile
from concourse.bass import ts, ds
from concourse.bass_utils import run_bass_kernel_spmd

BF16 = mybir.dt.bfloat16
FP32 = mybir.dt.float32
AF = mybir.ActivationFunctionType
ALU = mybir.AluOpType

S, B, D, H, DH, F = 2048, 2, 1024, 16, 64, 4096
NC = 8              # cores
CH = S // NC        # seq positions per core (256)
TQ = CH * B         # tokens per core (512)
P = 128
KT = D // P         # 8 k-tiles over D
MT = D // P         # 8 m-tiles over D
FT = F // P         # 32 tiles over F
TT = S // P         # 16 token-tiles per batch
LN_EPS = 1e-5
HP = H // 2         # 8 head-pairs


def _split_multiwaits(nc):
    # Walrus in this container encodes at most ONE sync-wait per instruction.
    # Tile's tail drain violates that; hoist extra waits onto fresh NoOps.
    for bb in nc.m.functions[0].blocks:
        new_insts = []
        for inst in bb.instructions:
            si = inst.sync_info
            if si is not None and si.on_wait and len(si.on_wait) > 1:
                waits = list(si.on_wait)
                for j, w in enumerate(waits[:-1]):
                    new_insts.append(mybir.InstNoOp(
                        name=f"{inst.name}-wsplit{j}", engine=inst.engine,
                        ins=[], outs=[],
                        sync_info=mybir.SyncInfo(on_wait=[w], on_update=[])))
                si.on_wait = [waits[-1]]
            new_insts.append(inst)
        bb.instructions = new_insts


def build_bass(split_waits=True, phases="ABCEF"):
    nc = bass.Bass(name="encoder_block", num_devices=NC, debug=False)

    # ---- I/O ----
    xT = nc.dram_tensor("xT", (D, B, S), BF16, kind="ExternalInput")
    xTq = nc.dram_tensor("xTq", (D, B, CH), BF16, kind="ExternalInput")
    xres = nc.dram_tensor("xres", (B, CH, D), FP32, kind="ExternalInput")
    wq = nc.dram_tensor("wq", (D, D), BF16, kind="ExternalInput")
    wk = nc.dram_tensor("wk", (D, D), BF16, kind="ExternalInput")
    wv = nc.dram_tensor("wv", (D, D), BF16, kind="ExternalInput")
    wo = nc.dram_tensor("wo", (D, D), BF16, kind="ExternalInput")
    w1 = nc.dram_tensor("w1", (D, F), BF16, kind="ExternalInput")
    w2 = nc.dram_tensor("w2", (F, D), BF16, kind="ExternalInput")
    identd = nc.dram_tensor("ident", (P, P), FP32, kind="ExternalInput")
    bqs = nc.dram_tensor("bqs", (D,), FP32, kind="ExternalInput")  # bq/8
    bk = nc.dram_tensor("bk", (D,), FP32, kind="ExternalInput")
    bb1 = nc.dram_tensor("bb1", (F,), FP32, kind="ExternalInput")
    bv_rep = nc.dram_tensor("bv_rep", (P, D), FP32, kind="ExternalInput")
    bo_rep = nc.dram_tensor("bo_rep", (P, D), FP32, kind="ExternalInput")
    bb2_rep = nc.dram_tensor("bb2_rep", (P, D), FP32, kind="ExternalInput")
    g1_rep = nc.dram_tensor("g1_rep", (P, D), FP32, kind="ExternalInput")
    b1_rep = nc.dram_tensor("b1_rep", (P, D), FP32, kind="ExternalInput")
    g2_rep = nc.dram_tensor("g2_rep", (P, D), FP32, kind="ExternalInput")
    b2_rep = nc.dram_tensor("b2_rep", (P, D), FP32, kind="ExternalInput")
    out = nc.dram_tensor("out", (B, CH, D), FP32, kind="ExternalOutput")

    xT_t = xT.rearrange("(kt p) b s -> p kt b s", p=P)
    xTq_t = xTq.rearrange("(kt p) b s -> p kt b s", p=P)
    xres_t = xres.rearrange("b (tq p) d -> p b tq d", p=P)
    out_t = out.rearrange("b (tq p) d -> p b tq d", p=P)
    wq_t = wq.rearrange("(kt p) n -> p kt n", p=P)
    wk_t = wk.rearrange("(kt p) n -> p kt n", p=P)
    wv_t = wv.rearrange("(kt p) n -> p kt n", p=P)
    wo_t = wo.rearrange("(kt p) n -> p kt n", p=P)
    w1_t = w1.rearrange("(kt p) n -> p kt n", p=P)
    w2_t = w2.rearrange("(kt p) n -> p kt n", p=P)
    bqs_t = bqs.rearrange("(m p) -> p m", p=P)
    bk_t = bk.rearrange("(m p) -> p m", p=P)
    bb1_t = bb1.rearrange("(m p) -> p m", p=P)

    eps_sb_box = []

    def layer_norm(tc, pool, t1, extra_sb, g_sb, bt_sb, dst):
        """dst = LN(t1 + extra)*g + bt.  t1 [P,D] fp32 SBUF is clobbered."""
        nc.vector.tensor_tensor(t1[:], t1[:], extra_sb[:], ALU.add)
        ssum = pool.tile([P, 1], FP32, tag="ln_sum")
        nc.vector.reduce_sum(ssum[:], t1[:], axis=mybir.AxisListType.X)
        negmean = pool.tile([P, 1], FP32, tag="ln_negmean")
        nc.scalar.mul(negmean[:], ssum[:], -1.0 / D)
        xc = pool.tile([P, D], FP32, tag="ln_xc")
        nc.vector.tensor_scalar_add(xc[:], t1[:], negmean[:])
        ss = pool.tile([P, 1], FP32, tag="ln_ss")
        nc.scalar.activation(t1[:], xc[:], AF.Square, accum_out=ss[:])
        st = pool.tile([P, 1], FP32, tag="ln_st")
        nc.scalar.activation(st[:], ss[:], AF.Sqrt,
                             bias=eps_sb_box[0][:], scale=1.0 / D)
        rstd = pool.tile([P, 1], FP32, tag="ln_rstd")
        nc.vector.reciprocal(rstd[:], st[:])
        nc.vector.tensor_scalar_mul(xc[:], xc[:], rstd[:])
        nc.vector.tensor_tensor(xc[:], xc[:], g_sb[:], ALU.mult)
        nc.vector.tensor_tensor(dst[:], xc[:], bt_sb[:], ALU.add)

    with tile.TileContext(nc) as tc:
        with (
            tc.tile_pool(name="persist", bufs=1) as pp,
            tc.tile_pool(name="dram", bufs=1, space="DRAM") as dpool,
        ):
            # alive for the whole kernel (~1.8 MB)
            ones_sb = pp.tile([P, DH], BF16, tag="ones")
            bqs_sb = pp.tile([P, MT], FP32, tag="bqs")
            bk_sb = pp.tile([P, MT], FP32, tag="bk")
            bb1_sb = pp.tile([P, FT], FP32, tag="bb1")

            kdram = dpool.tile([HP, P, B, S], BF16)               # K^T spill

            eps_sb = pp.tile([P, 1], FP32, tag="eps")
            eps_sb_box.append(eps_sb)
            nc.vector.memset(eps_sb[:], LN_EPS)
            nc.vector.memset(ones_sb[:], 1.0)
            nc.sync.dma_start(bqs_sb[:], bqs_t)
            nc.sync.dma_start(bk_sb[:], bk_t)
            nc.sync.dma_start(bb1_sb[:], bb1_t)

            with tc.tile_pool(name="x1p", bufs=1) as x1p:
                # alive A..F (3 MB)
                x1_sb = x1p.tile([P, B, B, D], FP32, tag="x1")
                x1T_sb = x1p.tile([P, KT, B, CH], BF16, tag="x1T")

                with tc.tile_pool(name="otx", bufs=1) as otx:
                    # alive A..C (1 MB)
                    oT_sb = otx.tile([P, MT, B, CH], BF16, tag="oT")

                    with tc.tile_pool(name="vq", bufs=1) as vq:
                        # alive A..B (~10 MB)
                        v_sb = vq.tile([P, B, TT, D], BF16, tag="v")
                        qT_sb = vq.tile([P, MT, B, CH], BF16, tag="qT")
                        bvr_sb = vq.tile([P, D], FP32, tag="bvr")
                        nc.sync.dma_start(bvr_sb[:], bv_rep[:])

                        # ===== Phase A: projections (K^T, V, Q^T) =====
                        with (
                            tc.tile_pool(name="wqkv", bufs=2) as wpool,
                            tc.tile_pool(name="xt", bufs=2) as xpool,
                            tc.tile_pool(name="aout", bufs=3) as apool,
                            tc.tile_pool(name="psA", bufs=4,
                                         space="PSUM") as psA,
                        ):
                            wk_sb = wpool.tile([P, KT, D], BF16, tag="w")
                            nc.sync.dma_start(wk_sb[:], wk_t)
                            wv_sb = wpool.tile([P, KT, D], BF16, tag="w")
                            nc.sync.dma_start(wv_sb[:], wv_t)
                            xtq_sb = xpool.tile([P, KT, B, CH], BF16,
                                                tag="xtq")
                            nc.sync.dma_start(xtq_sb[:], xTq_t)

                            SQ = S // 4  # 512-token stream chunks
                            for b in range(B):
                                for sh in range(4):
                                    xth = xpool.tile([P, KT, SQ], BF16,
                                                     tag="xth")
                                    nc.sync.dma_start(
                                        xth[:],
                                        xT_t[:, :, b, ds(sh * SQ, SQ)])
                                    # K^T -> DRAM spill (head-pair major)
                                    for m in range(MT):
                                        ps = psA.tile([P, 512], FP32,
                                                      tag="psa")
                                        for kt in range(KT):
                                            nc.tensor.matmul(
                                                ps[:],
                                                wk_sb[:, kt, ts(m, P)],
                                                xth[:, kt, :],
                                                start=(kt == 0),
                                                stop=(kt == KT - 1))
                                        ksb = apool.tile([P, 512], BF16,
                                                         tag="ksb")
                                        nc.vector.tensor_scalar_add(
                                            ksb[:], ps[:],
                                            bk_sb[:, ds(m, 1)])
                                        nc.sync.dma_start(
                                            kdram[m, :, b, ds(sh * SQ, SQ)],
                                            ksb[:])
                                    # V (token-major), bias added, in SBUF
                                    for tl in range(SQ // P):
                                        tt = sh * (SQ // P) + tl
                                        for nb in range(D // 512):
                                            ps = psA.tile([P, 512], FP32,
                                                          tag="psa")
                                            for kt in range(KT):
                                                nc.tensor.matmul(
                                                    ps[:],
                                                    xth[:, kt, ts(tl, P)],
                                                    wv_sb[:, kt, ts(nb, 512)],
                                                    start=(kt == 0),
                                                    stop=(kt == KT - 1))
                                            nc.vector.tensor_tensor(
                                                v_sb[:, b, tt, ts(nb, 512)],
                                                ps[:], bvr_sb[:, ts(nb, 512)],
                                                ALU.add)

                            # Q^T chunk, scaled by 1/sqrt(dh) (wq reuses
                            # a wqkv slot after wk is done)
                            wq_sb = wpool.tile([P, KT, D], BF16, tag="w")
                            nc.sync.dma_start(wq_sb[:], wq_t)
                            for b in range(B):
                                for m in range(MT):
                                    ps = psA.tile([P, CH], FP32, tag="psq")
                                    for kt in range(KT):
                                        nc.tensor.matmul(
                                            ps[:], wq_sb[:, kt, ts(m, P)],
                                            xtq_sb[:, kt, b, :],
                                            start=(kt == 0),
                                            stop=(kt == KT - 1))
                                    nc.vector.tensor_scalar(
                                        qT_sb[:, m, b, :], ps[:], 0.125,
                                        bqs_sb[:, ds(m, 1)],
                                        ALU.mult, ALU.add)

                        # ===== Phase B: attention =====
                        if "B" not in phases:
                            nc.vector.memset(oT_sb[:], 0.001)
                        with (
                            tc.tile_pool(name="kpair", bufs=2) as kpool,
                            tc.tile_pool(name="expst", bufs=2) as epool,
                            tc.tile_pool(name="battn", bufs=4) as bpool,
                            tc.tile_pool(name="psS", bufs=4,
                                         space="PSUM") as psS,
                            tc.tile_pool(name="psO", bufs=2,
                                         space="PSUM") as psO,
                            tc.tile_pool(name="psR", bufs=2,
                                         space="PSUM") as psR,
                        ):
                            for hp in range(HP if "B" in phases else 0):
                                kpair = kpool.tile([P, B, S], BF16,
                                                   tag="kpair")
                                nc.sync.dma_start(kpair[:], kdram[hp])
                                for b in range(B):
                                    for h01 in range(2):
                                        po = h01 * DH
                                        h = hp * 2 + h01
                                        expst = epool.tile([P, TT, CH], BF16,
                                                           tag="expst")
                                        for tt in range(TT):
                                            ps = psS.tile([P, CH], FP32,
                                                          tag="pss")
                                            nc.tensor.matmul(
                                                ps[:],
                                                kpair[ds(po, DH), b,
                                                      ts(tt, P)],
                                                qT_sb[ds(po, DH), hp, b, :])
                                            nc.scalar.activation(
                                                expst[:, tt, :], ps[:],
                                                AF.Exp)
                                        po_ps = psO.tile([DH, CH], FP32,
                                                         tag="pso")
                                        sum_ps = psR.tile([DH, CH], FP32,
                                                          tag="psum_r")
                                        for tt in range(TT):
                                            nc.tensor.matmul(
                                                po_ps[:],
                                                v_sb[:, b, tt,
                                                     ds(h * DH, DH)],
                                                expst[:, tt, :],
                                                start=(tt == 0),
                                                stop=(tt == TT - 1))
                                        for tt in range(TT):
                                            nc.tensor.matmul(
                                                sum_ps[:], ones_sb[:],
                                                expst[:, tt, :],
                                                start=(tt == 0),
                                                stop=(tt == TT - 1))
                                        rr = bpool.tile([DH, CH], FP32,
                                                        tag="rr")
                                        nc.vector.reciprocal(rr[:], sum_ps[:])
                                        nc.vector.tensor_tensor(
                                            oT_sb[ds(po, DH), hp, b, :],
                                            po_ps[:], rr[:], ALU.mult)

                    # ===== Phase C: O-proj + residual + LN1 (+ x1^T) =====
                    if "C" not in phases:
                        nc.vector.memset(x1_sb[:], 0.001)
                        nc.vector.memset(x1T_sb[:], 0.001)
                    with (
                        tc.tile_pool(name="wo_p", bufs=1) as wopool,
                        tc.tile_pool(name="cscr", bufs=2) as cpool,
                        tc.tile_pool(name="psC", bufs=2, space="PSUM") as psC,
                        tc.tile_pool(name="psD", bufs=2, space="PSUM") as psD,
                    ):
                        wo_sb = wopool.tile([P, KT, D], BF16, tag="wo")
                        nc.sync.dma_start(wo_sb[:], wo_t)
                        ident = wopool.tile([P, P], FP32, tag="ident")
                        nc.sync.dma_start(ident[:], identd[:])
                        bor_sb = wopool.tile([P, D], FP32, tag="bor")
                        g1r_sb = wopool.tile([P, D], FP32, tag="g1r")
                        b1r_sb = wopool.tile([P, D], FP32, tag="b1r")
                        nc.sync.dma_start(bor_sb[:], bo_rep[:])
                        nc.sync.dma_start(g1r_sb[:], g1_rep[:])
                        nc.sync.dma_start(b1r_sb[:], b1_rep[:])
                        for b in range(B if "C" in phases else 0):
                            for tq in range(B):
                                ps = psC.tile([P, D], FP32, tag="psc")
                                for nb in range(D // 512):
                                    for kt in range(KT):
                                        nc.tensor.matmul(
                                            ps[:, ts(nb, 512)],
                                            oT_sb[:, kt, b, ts(tq, P)],
                                            wo_sb[:, kt, ts(nb, 512)],
                                            start=(kt == 0),
                                            stop=(kt == KT - 1))
                                t1 = cpool.tile([P, D], FP32, tag="c_t1")
                                nc.vector.tensor_tensor(
                                    t1[:], ps[:], bor_sb[:], ALU.add)
                                xres_sb = cpool.tile([P, D], FP32, tag="xres")
                                nc.sync.dma_start(xres_sb[:],
                                                  xres_t[:, b, tq, :])
                                layer_norm(tc, cpool, t1, xres_sb,
                                           g1r_sb, b1r_sb,
                                           x1_sb[:, b, tq, :])
                                for kd in range(KT):
                                    pt = psD.tile([P, P], FP32, tag="psd")
                                    nc.tensor.transpose(
                                        pt[:], x1_sb[:, b, tq, ts(kd, P)],
                                        ident[:])
                                    nc.scalar.copy(
                                        x1T_sb[:, kd, b, ts(tq, P)], pt[:])

                # ===== Phase E: FFN1  hT = relu(x1@W1+bb1)^T =====
                with tc.tile_pool(name="hT", bufs=1) as hpool:
                    hT_sb = hpool.tile([P, FT, TQ], BF16, tag="hT")
                    with (
                        tc.tile_pool(name="w1_p", bufs=1) as w1pool,
                        tc.tile_pool(name="psE", bufs=4,
                                     space="PSUM") as psE,
                    ):
                        w1_sb = w1pool.tile([P, KT, F], BF16, tag="w1")
                        nc.sync.dma_start(w1_sb[:], w1_t)
                        if "E" not in phases:
                            nc.vector.memset(hT_sb[:], 0.001)
                        for mh in range(FT if "E" in phases else 0):
                            ps = psE.tile([P, TQ], FP32, tag="pse")
                            for kt in range(KT):
                                nc.tensor.matmul(
                                    ps[:], w1_sb[:, kt, ts(mh, P)],
                                    x1T_sb[:, kt, :, :],
                                    start=(kt == 0), stop=(kt == KT - 1))
                            nc.scalar.activation(
                                hT_sb[:, mh, :], ps[:], AF.Relu,
                                bias=bb1_sb[:, ds(mh, 1)])

                    # ===== Phase F: FFN2 + residual + LN2 -> out =====
                    with (
                        tc.tile_pool(name="w2_p", bufs=1) as w2pool,
                        tc.tile_pool(name="fscr", bufs=2) as fpool,
                        tc.tile_pool(name="psF", bufs=2, space="PSUM") as psF,
                    ):
                        w2_sb = w2pool.tile([P, FT, D], BF16, tag="w2")
                        nc.sync.dma_start(w2_sb[:], w2_t)
                        bb2r_sb = w2pool.tile([P, D], FP32, tag="bb2r")
                        g2r_sb = w2pool.tile([P, D], FP32, tag="g2r")
                        b2r_sb = w2pool.tile([P, D], FP32, tag="b2r")
                        nc.sync.dma_start(bb2r_sb[:], bb2_rep[:])
                        nc.sync.dma_start(g2r_sb[:], g2_rep[:])
                        nc.sync.dma_start(b2r_sb[:], b2_rep[:])
                        if "F" not in phases:
                            for b in range(B):
                                for tq in range(B):
                                    dummy = fpool.tile([P, D], FP32,
                                                       tag="f_out")
                                    nc.vector.memset(dummy[:], 0.5)
                                    nc.sync.dma_start(out_t[:, b, tq, :],
                                                      dummy[:])
                        for b in range(B if "F" in phases else 0):
                            for tq in range(B):
                                c = b * B + tq
                                ps = psF.tile([P, D], FP32, tag="psf")
                                for nb in range(D // 512):
                                    for kt in range(FT):
                                        nc.tensor.matmul(
                                            ps[:, ts(nb, 512)],
                                            hT_sb[:, kt, ts(c, P)],
                                            w2_sb[:, kt, ts(nb, 512)],
                                            start=(kt == 0),
                                            stop=(kt == FT - 1))
                                t1 = fpool.tile([P, D], FP32, tag="f_t1")
                                nc.vector.tensor_tensor(
                                    t1[:], ps[:], bb2r_sb[:], ALU.add)
                                o_sb = fpool.tile([P, D], FP32, tag="f_out")
                                layer_norm(tc, fpool, t1,
                                           x1_sb[:, b, tq, :],
                                           g2r_sb, b2r_sb, o_sb)
                                nc.sync.dma_start(out_t[:, b, tq, :],
                                                  o_sb[:])

    if split_waits:
        _split_multiwaits(nc)
    return nc


_NC_CACHE = None


def _get_bass():
    global _NC_CACHE
    if _NC_CACHE is None:
        _NC_CACHE = build_bass()
    return _NC_CACHE


def make_in_maps(x, Wq, bq, Wk, bk, Wv, bv, Wo, bo, g1, b1, W1, bb1, W2, bb2,
                 g2, b2):
    bf = ml_dtypes.bfloat16
    x = np.asarray(x, np.float32)
    xT = np.ascontiguousarray(x.transpose(2, 1, 0)).astype(bf)   # [D,B,S]
    shared = {
        "xT": xT,
        "wq": np.asarray(Wq, np.float32).astype(bf),
        "wk": np.asarray(Wk, np.float32).astype(bf),
        "wv": np.asarray(Wv, np.float32).astype(bf),
        "wo": np.asarray(Wo, np.float32).astype(bf),
        "w1": np.asarray(W1, np.float32).astype(bf),
        "w2": np.asarray(W2, np.float32).astype(bf),
        "ident": np.eye(P, dtype=np.float32),
        "bqs": (np.asarray(bq, np.float32) / 8.0),
        "bk": np.asarray(bk, np.float32),
        "bb1": np.asarray(bb1, np.float32),
        "bv_rep": np.tile(np.asarray(bv, np.float32), (P, 1)),
        "bo_rep": np.tile(np.asarray(bo, np.float32), (P, 1)),
        "bb2_rep": np.tile(np.asarray(bb2, np.float32), (P, 1)),
        "g1_rep": np.tile(np.asarray(g1, np.float32), (P, 1)),
        "b1_rep": np.tile(np.asarray(b1, np.float32), (P, 1)),
        "g2_rep": np.tile(np.asarray(g2, np.float32), (P, 1)),
        "b2_rep": np.tile(np.asarray(b2, np.float32), (P, 1)),
    }
    in_maps = []
    for c in range(NC):
        sl = slice(c * CH, (c + 1) * CH)
        m = dict(shared)
        m["xTq"] = np.ascontiguousarray(xT[:, :, sl])
        m["xres"] = np.ascontiguousarray(
            x[sl].transpose(1, 0, 2))              # [B, CH, D]
        in_maps.append(m)
    return in_maps


def assemble(results):
    out = np.empty((S, B, D), np.float32)
    for c, r in enumerate(results):
        out[c * CH:(c + 1) * CH] = r["out"].transpose(1, 0, 2)
    return out


def kernel(**inputs) -> np.ndarray:
    nc = _get_bass()
    in_maps = make_in_maps(**inputs)
    res = run_bass_kernel_spmd(nc, in_maps, core_ids=list(range(NC)))
    return assemble(res.results)

